# revision 38
# baseline (speedup 1.0000x reference)
"""Trainium2 Bass kernel for nn_AlignModule (QAConv correlation + PAM).

Reference computation (B=32, C=512, H=24, W=8, hw=192, C8=64):
  xf = x.reshape(B, C, hw)
  score[g,p,n,m] = sum_c xf[g,c,m] * xf[p,c,n]          # [B,B,hw,hw]
  kernel_max[g,p,n] = max_m score[g,p,n,m]              # [B,B,hw]
  q = Wq @ xf[b] + bq; k = Wk @ xf[b] + bk              # [B,C8,hw]
  energy[b,m,n] = sum_q q[b,q,m] k[b,q,n]
  pos_max[b,m] = max_n energy[b,m,n]                    # [B,hw]
  out = concat([kernel_max, pos_max[None]], axis=0)     # [B+1,B,hw]

Sharding: data-parallel over g across 8 cores (4 images per core).

Speed strategy ("sum/diff" pair-max, vs the 43.8us fp8 baseline):
 - Only Act and DVE can read PSUM (one PSUM operand/instruction), so
   the 4.7M-element psum max-reduce is the wall. Halve it in the PE:
   max(s0,s1) = (s0+s1)/2 + |s0-s1|/2, and both halves are LINEAR in
   the kernel columns - the host uploads paired-sum and paired-diff
   moving columns (fp8 hi+lo), so each 2-j generation fills a u-psum
   and a d-psum of [128, 2, 384] each. Act casts |d| (Abs -> fp16),
   the PE adds it back into the u-psum with an fp16 identity matmul,
   and the u-psum then holds the 96-wide PAIR MAXES: one DVE
   reduce_max (925ns) or an Act cast + short DVE funnel finishes the
   gen. Per generation the drain costs ~825ns Act + ~925ns DVE
   instead of the ~1465 Act + ~1000 DVE an unpaired gen needs.
 - Gen variants balance engines: V1 idadd+DVE-reduce (most), V2
   idadd+Act-cast+DVE-funnel (Act spill), V3 no-idadd, DVE stt-add +
   funnel (PE relief). Every gen feeds both Act and DVE, so both ramp
   right after the first generation lands.
 - PSUM rings: u-tag and d-tag, [128,2,512] x 2 bufs each (8 banks).
   Iteration k emits: mms(k), idadd(k-1)+drain(k-1), e-cast(k) - so
   each ring's reader is always emitted before its buffer re-allocs
   and the PE never stalls on the Act cast of the gen it just filled.
   PAM projection/energy gens ride the d-ring (same-iteration drains).
 - QAConv matmuls fp8(e4m3) DoubleRow; g==p diagonal tiles accumulate
   hi*lo + lo*hi cross-terms on both operand sides.
 - Outputs stream out in chunks as their j-blocks settle.
"""

import numpy as np
import ml_dtypes

import concourse.mybir as mybir
import concourse.tile as tile
from concourse import bacc
from concourse.bass_utils import run_bass_kernel_spmd

B = 32
C = 512
HW = 192
C8 = 64
N_CORES = 8
GPC = B // N_CORES            # images per core (4)
FLAT = B * HW                 # flattened (p, n) axis (6144)
NJ = FLAT // 128              # stationary 128-column blocks (48)
GROLL = GPC * HW              # per-core roll step (768)
NCH = FLAT // GROLL           # x column chunks (8)
JPC = GROLL // 128            # j blocks per column chunk (6)

F32 = mybir.dt.float32
F16 = mybir.dt.float16
F8 = mybir.dt.float8e4
AX_X = mybir.AxisListType.X
DR = mybir.MatmulPerfMode.DoubleRow
IDENT = mybir.ActivationFunctionType.Identity
ABS = mybir.ActivationFunctionType.Abs
BP = mybir.AluOpType.bypass
ADD = mybir.AluOpType.add

# tiles (j, gp) that contain g==p diagonal blocks (rolled layout puts
# the core's own images at columns [0, 768) -> j 0..5)
DIAG_TILES = {(0, 0), (1, 0), (2, 0), (3, 1), (4, 1), (5, 1)}
# diag cross-term moving-column window (gen-local offset, width): tiles
# whose 128 stationary n's sit inside one image only need that image's
# 96 pair-columns
DIAG_MO = {(0, 0): (0, 96), (1, 0): (0, 192), (2, 0): (96, 96),
           (3, 1): (192, 96), (4, 1): (192, 192), (5, 1): (288, 96)}

# generation order: chunk-1 js first (earliest DMAs), diag js once
# xlo/xmlo land, then chunks 2-7.
GEN_JS = [(6, 7), (8, 9), (10, 11), (12, 13), (0, 1), (2, 3), (4, 5)] + [
    (j, j + 1) for j in range(14, NJ, 2)
]
# drain variant per gen: 1 = idadd + DVE reduce, 2 = idadd + Act cast +
# DVE funnel (Act spill), 3 = no idadd, DVE stt-add + funnel (PE relief)
V2_GENS = set()
V3_GENS = {10, 16, 20}

_COMPILED = None


def _build():
    nc = bacc.Bacc("TRN2", target_bir_lowering=False, debug=False)

    xr = nc.dram_tensor("xr", [C, FLAT], F8, kind="ExternalInput").ap()
    xlo = nc.dram_tensor("xlo", [C, GROLL], F8, kind="ExternalInput").ap()
    xm = nc.dram_tensor("xm", [C, GROLL], F8, kind="ExternalInput").ap()
    xmlo = nc.dram_tensor("xmlo", [C, GROLL], F8, kind="ExternalInput").ap()
    w = nc.dram_tensor("w", [C, 2 * C8], F8, kind="ExternalInput").ap()
    bqk = nc.dram_tensor("bqk", [2 * C8, 1], F32, kind="ExternalInput").ap()
    ident = nc.dram_tensor("ident", [128, 128], F16, kind="ExternalInput").ap()
    kres = nc.dram_tensor("kres", [128, NJ, 2, 2], F16, kind="ExternalOutput").ap()
    pam = nc.dram_tensor("pam", [128, GPC, 2], F16, kind="ExternalOutput").ap()

    xrr = xr.rearrange("(co p) f -> p co f", p=128)
    with tile.TileContext(nc) as tc:
        with (
            tc.tile_pool(name="sb", bufs=1) as sb,
            tc.tile_pool(name="epool", bufs=4) as epool,
            tc.tile_pool(name="cpool", bufs=2) as cpool,
            tc.tile_pool(name="tpool", bufs=2) as tpool,
            tc.tile_pool(name="upsum", bufs=2, space="PSUM") as upsum,
            tc.tile_pool(name="dpsum", bufs=2, space="PSUM") as dpsum,
        ):
            # ---- input DMAs (single SP queue, priority order) ----
            xm_sb = sb.tile([128, 4, GROLL], F8, tag="xm", name="xm_sb")
            nc.sync.dma_start(xm_sb[:], xm.rearrange("(co p) f -> p co f", p=128))
            xc = [None] * NCH
            for c in (1, 0):
                t = sb.tile([128, 4, GROLL], F8, tag=f"x{c}", name=f"x{c}")
                nc.sync.dma_start(t[:], xrr[:, :, c * GROLL:(c + 1) * GROLL])
                xc[c] = t
            w_sb = sb.tile([128, 4, 2 * C8], F8, tag="w", name="w_sb")
            nc.sync.dma_start(w_sb[:], w.rearrange("(co p) q -> p co q", p=128))
            ident_sb = sb.tile([128, 128], F16, tag="ident", name="ident_sb")
            nc.sync.dma_start(ident_sb[:], ident[:])
            bqk_sb = sb.tile([2 * C8, 1], F32, tag="bqk", name="bqk_sb")
            nc.sync.dma_start(bqk_sb[:], bqk[:])
            xlo_sb = sb.tile([128, 4, GROLL], F8, tag="xlo", name="xlo_sb")
            nc.sync.dma_start(xlo_sb[:], xlo.rearrange("(co p) f -> p co f", p=128))
            xmlo_sb = sb.tile([128, 4, GROLL], F8, tag="xmlo", name="xmlo_sb")
            nc.sync.dma_start(xmlo_sb[:], xmlo.rearrange("(co p) f -> p co f", p=128))
            for c in range(2, NCH):
                t = sb.tile([128, 4, GROLL], F8, tag=f"x{c}", name=f"x{c}")
                nc.sync.dma_start(t[:], xrr[:, :, c * GROLL:(c + 1) * GROLL])
                xc[c] = t

            # ---- persistent sbuf ----
            qk_sb = sb.tile([C8, 4, 2 * HW], F16, tag="qk", name="qk_sb")
            res_sb = sb.tile([128, NJ, 2, 2], F16, tag="res", name="res_sb")
            pam_sb = sb.tile([128, GPC, 2], F16, tag="pam", name="pam_sb")

            # ---- PE warmup (p-state ramp during DMA wait) ----
            warm_sb = sb.tile([128, 2, 2 * HW], F8, tag="warm", name="warm_sb")
            nc.gpsimd.memset(warm_sb[:], 0.0)
            wt = dpsum.tile([128, 2, 512], F32, tag="d", name="warm_ps")
            for wi in range(4):
                nc.tensor.matmul(
                    wt[:, wi % 2, 0:2 * HW], warm_sb[:, :, 0:128],
                    warm_sb[:], start=True, stop=True, perf_mode=DR,
                )

            def side_matmuls(pt, jsl, j0, j1, side, close):
                """u- or d-side matmuls for gen (j0, j1) into pt slots
                jsl. ONE start group per psum bank (a second start=True
                in a bank zeroes it): kt0 full-width starts, diag
                cross-terms accumulate sub-regions, kt1 stops."""
                for jx, j in enumerate((j0, j1)):
                    cc, jp = divmod(j, JPC)
                    out = pt[:, jsl[jx], 0:2 * HW]
                    mo = side * (GROLL // 2)
                    nc.tensor.matmul(
                        out, xc[cc][:, 0:2, jp * 128:(jp + 1) * 128],
                        xm_sb[:, 0:2, mo:mo + 2 * HW],
                        start=True, stop=False, perf_mode=DR)
                    for gp in range(2):
                        if (j, gp) not in DIAG_TILES:
                            continue
                        off, wid = DIAG_MO[(j, gp)]
                        sub = pt[:, jsl[jx], off:off + wid]
                        for lt, rt in ((xlo_sb, xm_sb), (xc[0], xmlo_sb)):
                            for kt in range(2):
                                nc.tensor.matmul(
                                    sub,
                                    lt[:, 2 * kt:2 * kt + 2,
                                       jp * 128:(jp + 1) * 128],
                                    rt[:, 2 * kt:2 * kt + 2,
                                       mo + off:mo + off + wid],
                                    start=False, stop=False,
                                    perf_mode=DR)
                    nc.tensor.matmul(
                        out, xc[cc][:, 2:4, jp * 128:(jp + 1) * 128],
                        xm_sb[:, 2:4, mo:mo + 2 * HW],
                        start=False, stop=close, perf_mode=DR)

            def d4(pt):
                return pt[:, :, 0:2 * HW].rearrange(
                    "p j (g s m) -> p j g s m", g=2, s=2)

            def e_cast(dt, j0):
                """Act: |d| -> fp16 sbuf."""
                et = epool.tile([128, 2, 2, 2, 96], F16, tag="e",
                                name=f"e_{j0}")
                nc.scalar.activation(et[:], d4(dt), ABS)
                return et

            def idadd(ut, et):
                """PE: u += |d| via identity matmul; closes the u banks."""
                for jx in range(2):
                    nc.tensor.matmul(
                        ut[:, jx, 0:2 * HW], ident_sb[:],
                        et[:, jx], start=False, stop=True,
                    )

            def funnel(src, j0, tag):
                """DVE max-funnel over one gen's 96-wide pair maxes."""
                t1 = tpool.tile([128, 2, 2, 2, 48], F16, tag="t1",
                                name=f"t1_{tag}{j0}")
                nc.vector.tensor_max(t1[:], src[:, :, :, :, 0:48],
                                     src[:, :, :, :, 48:96])
                t2 = tpool.tile([128, 2, 2, 2, 24], F16, tag="t2",
                                name=f"t2_{tag}{j0}")
                nc.vector.tensor_max(t2[:], t1[:, :, :, :, 0:24],
                                     t1[:, :, :, :, 24:48])
                t3 = tpool.tile([128, 2, 2, 2, 12], F16, tag="t3",
                                name=f"t3_{tag}{j0}")
                nc.vector.tensor_max(t3[:], t2[:, :, :, :, 0:12],
                                     t2[:, :, :, :, 12:24])
                nc.vector.reduce_max(res_sb[:, j0:j0 + 2], t3[:], axis=AX_X)

            def drain(gi, ut, et, j0):
                if gi in V3_GENS:
                    # no idadd was emitted: pairmax = u + e on DVE, funnel
                    pm = cpool.tile([128, 2, 2, 2, 96], F16, tag="uc",
                                    name=f"pm_{j0}")
                    nc.vector.scalar_tensor_tensor(
                        pm[:], d4(ut), 0.0, et[:], op0=BP, op1=ADD)
                    funnel(pm, j0, "v3")
                elif gi in V2_GENS:
                    uc = cpool.tile([128, 2, 2, 2, 96], F16, tag="uc",
                                    name=f"uc_{j0}")
                    nc.scalar.activation(uc[:], d4(ut), IDENT)
                    funnel(uc, j0, "v2")
                else:
                    nc.vector.reduce_max(res_sb[:, j0:j0 + 2], d4(ut),
                                         axis=AX_X)

            # ---- PAM (psum gens ride the d-ring) ----
            def pam_proj(qi):
                pt = dpsum.tile([128, 2, 512], F32, tag="d", name=f"proj{qi}")
                for gp in range(2):
                    for kt in range(2):
                        nc.tensor.matmul(
                            pt[0:C8, gp, 0:2 * HW],
                            w_sb[:, 2 * kt:2 * kt + 2,
                                 qi * C8:(qi + 1) * C8],
                            xc[0][:, 2 * kt:2 * kt + 2,
                                  gp * 2 * HW:(gp + 1) * 2 * HW],
                            start=(kt == 0), stop=(kt == 1), perf_mode=DR,
                        )
                nc.scalar.activation(
                    qk_sb[:, 2 * qi:2 * qi + 2], pt[0:C8, :, 0:2 * HW],
                    IDENT, bias=bqk_sb[qi * C8:(qi + 1) * C8])

            def pam_energy(h):
                """energy for images b = 2h, 2h+1 in one d-ring gen.
                Each bank is zero-initialized by one full-width matmul on
                the zeroed warm tile (one start group per bank), then the
                two m-chunk matmuls accumulate their sub-regions."""
                et = dpsum.tile([128, 2, 512], F32, tag="d", name=f"en{h}")
                for bb in range(2):
                    b = 2 * h + bb
                    gp, s = divmod(b, 2)
                    qb = qk_sb[:, 2 * 0 + gp, s * HW:(s + 1) * HW]
                    kb = qk_sb[:, 2 * 1 + gp, s * HW:(s + 1) * HW]
                    nc.tensor.matmul(
                        et[:, bb, 0:2 * HW], warm_sb[:, :, 0:128],
                        warm_sb[:], start=True, stop=False, perf_mode=DR)
                    for mch, m0 in enumerate((0, 64)):
                        nc.tensor.matmul(
                            et[:, bb, mch * HW:mch * HW + HW],
                            qb[:, m0:m0 + 128], kb[:],
                            start=False, stop=(mch == 1),
                        )
                nc.vector.reduce_max(
                    pam_sb[:, 2 * h:2 * h + 2],
                    et[:, :, 0:2 * HW].rearrange("p b (s n) -> p b s n", s=2),
                    axis=AX_X)

            # ---- emission schedule ----
            # Iteration k emits: mms(k), idadd(k-1)+drain(k-1), e-cast(k),
            # pam hook. Rings: u_k reuses u_{k-2} (drained in iter k-1);
            # d_k reuses d_{k-2} (e-cast emitted in iter k-2).
            pend = None   # (gi, ut, et, j0)
            for gi, (j0, j1) in enumerate(GEN_JS):
                dt = dpsum.tile([128, 2, 512], F32, tag="d", name=f"d_{j0}")
                side_matmuls(dt, (0, 1), j0, j1, 1, True)
                et = e_cast(dt, j0)
                ut = upsum.tile([128, 2, 512], F32, tag="u", name=f"u_{j0}")
                side_matmuls(ut, (0, 1), j0, j1, 0, gi in V3_GENS)
                if pend is not None:
                    pgi, put, pet, pj0 = pend
                    if pgi not in V3_GENS:
                        idadd(put, pet)
                    drain(pgi, put, pet, pj0)
                pend = (gi, ut, et, j0)
                if gi == 1:
                    pam_proj(0)
                elif gi == 2:
                    pam_proj(1)
                elif gi == 5:
                    pam_energy(0)
                elif gi == 6:
                    pam_energy(1)
                elif gi == 8:
                    nc.sync.dma_start(pam[:], pam_sb[:])
                elif gi == 10:
                    # gens 0-9 drained: js 6-13, 0-5, 14-19
                    nc.sync.dma_start(kres[:, 0:20], res_sb[:, 0:20])
                elif gi == 16:
                    nc.sync.dma_start(kres[:, 20:32], res_sb[:, 20:32])
                elif gi == 22:
                    nc.sync.dma_start(kres[:, 32:44], res_sb[:, 32:44])
            pgi, put, pet, pj0 = pend
            if pgi not in V3_GENS:
                idadd(put, pet)
            drain(pgi, put, pet, pj0)
            nc.sync.dma_start(kres[:, 44:NJ], res_sb[:, 44:NJ])

    nc.compile()
    return nc


def kernel(x, Wq, bq, Wk, bk):
    global _COMPILED
    if _COMPILED is None:
        _COMPILED = _build()
    nc = _COMPILED

    x = np.ascontiguousarray(x, dtype=np.float32)
    xT = x.reshape(B, C, HW).transpose(1, 0, 2).reshape(C, FLAT)
    xT8 = np.ascontiguousarray(xT).astype(ml_dtypes.float8_e4m3)
    xT8f = xT8.astype(np.float32)
    w2 = np.concatenate([np.asarray(Wq, np.float32).T,
                         np.asarray(Wk, np.float32).T], axis=1)
    w8 = np.ascontiguousarray(w2).astype(ml_dtypes.float8_e4m3)
    bqk2 = np.concatenate([np.asarray(bq, np.float32),
                           np.asarray(bk, np.float32)]).reshape(2 * C8, 1)
    bqk2 = np.ascontiguousarray(bqk2)
    id16 = np.eye(128, dtype=np.float16)

    in_maps = []
    for i in range(N_CORES):
        own = xT[:, i * GROLL:(i + 1) * GROLL].reshape(C, GPC, HW // 2, 2)
        xs = (own[..., 0] + own[..., 1]) * 0.5        # [C, 4, 96]
        xd = (own[..., 0] - own[..., 1]) * 0.5
        xmf = np.concatenate(
            [xs.reshape(C, GROLL // 2), xd.reshape(C, GROLL // 2)], axis=1)
        xm8 = xmf.astype(ml_dtypes.float8_e4m3)
        xmlo8 = (xmf - xm8.astype(np.float32)).astype(ml_dtypes.float8_e4m3)
        in_maps.append({
            "xr": np.ascontiguousarray(np.roll(xT8, -i * GROLL, axis=1)),
            "xlo": np.ascontiguousarray(
                xT[:, i * GROLL:(i + 1) * GROLL]
                - xT8f[:, i * GROLL:(i + 1) * GROLL]
            ).astype(ml_dtypes.float8_e4m3),
            "xm": np.ascontiguousarray(xm8),
            "xmlo": np.ascontiguousarray(xmlo8),
            "w": w8,
            "bqk": bqk2,
            "ident": id16,
        })

    res = run_bass_kernel_spmd(nc, in_maps, core_ids=list(range(N_CORES)))

    kernel_max = np.empty((B, FLAT), np.float32)
    pos_max = np.empty((B, HW), np.float32)
    for i, r in enumerate(res.results):
        kr = np.asarray(r["kres"]).astype(np.float32)   # [128, NJ, 2, 2]
        arr = kr.transpose(2, 3, 1, 0).reshape(GPC, FLAT)
        for gl in range(GPC):
            kernel_max[i * GPC + gl] = np.roll(arr[gl], i * GROLL)
        pm = np.asarray(r["pam"]).astype(np.float32)    # [128, 4, 2]
        for b in range(GPC):
            pos_max[i * GPC + b, 0:128] = pm[:, b, 0]
            pos_max[i * GPC + b, 128:HW] = pm[64:128, b, 1]

    return np.concatenate(
        [kernel_max.reshape(B, B, HW), pos_max[None]], axis=0
    ).astype(np.float32)


# revision 39
# speedup vs baseline: 1.0316x; 1.0316x over previous
"""Trainium2 Bass kernel for nn_AlignModule (QAConv correlation + PAM).

Reference computation (B=32, C=512, H=24, W=8, hw=192, C8=64):
  xf = x.reshape(B, C, hw)
  score[g,p,n,m] = sum_c xf[g,c,m] * xf[p,c,n]          # [B,B,hw,hw]
  kernel_max[g,p,n] = max_m score[g,p,n,m]              # [B,B,hw]
  q = Wq @ xf[b] + bq; k = Wk @ xf[b] + bk              # [B,C8,hw]
  energy[b,m,n] = sum_q q[b,q,m] k[b,q,n]
  pos_max[b,m] = max_n energy[b,m,n]                    # [B,hw]
  out = concat([kernel_max, pos_max[None]], axis=0)     # [B+1,B,hw]

Sharding: data-parallel over g across 8 cores (4 images per core).

Speed strategy ("sum/diff" pair-max, vs the 43.8us fp8 baseline):
 - Only Act and DVE can read PSUM (one PSUM operand/instruction), so
   the 4.7M-element psum max-reduce is the wall. Halve it in the PE:
   max(s0,s1) = (s0+s1)/2 + |s0-s1|/2, and both halves are LINEAR in
   the kernel columns - the host uploads paired-sum and paired-diff
   moving columns (fp8 hi+lo), so each 2-j generation fills a u-psum
   and a d-psum of [128, 2, 384] each. Act casts |d| (Abs -> fp16),
   the PE adds it back into the u-psum with an fp16 identity matmul,
   and the u-psum then holds the 96-wide PAIR MAXES: one DVE
   reduce_max (925ns) or an Act cast + short DVE funnel finishes the
   gen. Per generation the drain costs ~825ns Act + ~925ns DVE
   instead of the ~1465 Act + ~1000 DVE an unpaired gen needs.
 - Gen variants balance engines: V1 idadd+DVE-reduce (most), V2
   idadd+Act-cast+DVE-funnel (Act spill), V3 no-idadd, DVE stt-add +
   funnel (PE relief). Every gen feeds both Act and DVE, so both ramp
   right after the first generation lands.
 - PSUM rings: u-tag and d-tag, [128,2,512] x 2 bufs each (8 banks).
   Iteration k emits: mms(k), idadd(k-1)+drain(k-1), e-cast(k) - so
   each ring's reader is always emitted before its buffer re-allocs
   and the PE never stalls on the Act cast of the gen it just filled.
   PAM projection/energy gens ride the d-ring (same-iteration drains).
 - QAConv matmuls fp8(e4m3) DoubleRow; g==p diagonal tiles accumulate
   hi*lo + lo*hi cross-terms on both operand sides.
 - Outputs stream out in chunks as their j-blocks settle.
"""

import numpy as np
import ml_dtypes

import concourse.mybir as mybir
import concourse.tile as tile
from concourse import bacc
from concourse.bass_utils import run_bass_kernel_spmd

B = 32
C = 512
HW = 192
C8 = 64
N_CORES = 8
GPC = B // N_CORES            # images per core (4)
FLAT = B * HW                 # flattened (p, n) axis (6144)
NJ = FLAT // 128              # stationary 128-column blocks (48)
GROLL = GPC * HW              # per-core roll step (768)
NCH = FLAT // GROLL           # x column chunks (8)
JPC = GROLL // 128            # j blocks per column chunk (6)

F32 = mybir.dt.float32
F16 = mybir.dt.float16
F8 = mybir.dt.float8e4
AX_X = mybir.AxisListType.X
DR = mybir.MatmulPerfMode.DoubleRow
IDENT = mybir.ActivationFunctionType.Identity
ABS = mybir.ActivationFunctionType.Abs
BP = mybir.AluOpType.bypass
ADD = mybir.AluOpType.add

# tiles (j, gp) that contain g==p diagonal blocks (rolled layout puts
# the core's own images at columns [0, 768) -> j 0..5)
DIAG_TILES = {(0, 0), (1, 0), (2, 0), (3, 1), (4, 1), (5, 1)}
# diag cross-term moving-column window (gen-local offset, width): tiles
# whose 128 stationary n's sit inside one image only need that image's
# 96 pair-columns
DIAG_MO = {(0, 0): (0, 96), (1, 0): (0, 192), (2, 0): (96, 96),
           (3, 1): (192, 96), (4, 1): (192, 192), (5, 1): (288, 96)}

# generation order: chunk-1 js first (earliest DMAs), diag js once
# xlo/xmlo land, then chunks 2-7.
GEN_JS = [(6, 7), (8, 9), (10, 11), (12, 13), (0, 1), (2, 3), (4, 5)] + [
    (j, j + 1) for j in range(14, NJ, 2)
]
# drain variant per gen: 1 = idadd + DVE reduce, 2 = idadd + Act cast +
# DVE funnel (Act spill), 3 = no idadd, DVE stt-add + funnel (PE relief)
V2_GENS = set()
V3_GENS = {10, 16}

_COMPILED = None


def _build():
    nc = bacc.Bacc("TRN2", target_bir_lowering=False, debug=False)

    xr = nc.dram_tensor("xr", [C, FLAT], F8, kind="ExternalInput").ap()
    xlo = nc.dram_tensor("xlo", [C, GROLL], F8, kind="ExternalInput").ap()
    xm = nc.dram_tensor("xm", [C, GROLL], F8, kind="ExternalInput").ap()
    xmlo = nc.dram_tensor("xmlo", [C, GROLL], F8, kind="ExternalInput").ap()
    w = nc.dram_tensor("w", [C, 2 * C8], F8, kind="ExternalInput").ap()
    bqk = nc.dram_tensor("bqk", [2 * C8, 1], F32, kind="ExternalInput").ap()
    ident = nc.dram_tensor("ident", [128, 128], F16, kind="ExternalInput").ap()
    kres = nc.dram_tensor("kres", [128, NJ, 2, 2], F16, kind="ExternalOutput").ap()
    pam = nc.dram_tensor("pam", [128, GPC, 2], F16, kind="ExternalOutput").ap()

    xrr = xr.rearrange("(co p) f -> p co f", p=128)
    with tile.TileContext(nc) as tc:
        with (
            tc.tile_pool(name="sb", bufs=1) as sb,
            tc.tile_pool(name="epool", bufs=4) as epool,
            tc.tile_pool(name="cpool", bufs=2) as cpool,
            tc.tile_pool(name="tpool", bufs=2) as tpool,
            tc.tile_pool(name="upsum", bufs=2, space="PSUM") as upsum,
            tc.tile_pool(name="dpsum", bufs=2, space="PSUM") as dpsum,
        ):
            # ---- input DMAs (single SP queue, priority order) ----
            xm_sb = sb.tile([128, 4, GROLL], F8, tag="xm", name="xm_sb")
            nc.sync.dma_start(xm_sb[:], xm.rearrange("(co p) f -> p co f", p=128))
            xc = [None] * NCH
            for c in (1, 0):
                t = sb.tile([128, 4, GROLL], F8, tag=f"x{c}", name=f"x{c}")
                nc.sync.dma_start(t[:], xrr[:, :, c * GROLL:(c + 1) * GROLL])
                xc[c] = t
            w_sb = sb.tile([128, 4, 2 * C8], F8, tag="w", name="w_sb")
            nc.sync.dma_start(w_sb[:], w.rearrange("(co p) q -> p co q", p=128))
            ident_sb = sb.tile([128, 128], F16, tag="ident", name="ident_sb")
            nc.sync.dma_start(ident_sb[:], ident[:])
            bqk_sb = sb.tile([2 * C8, 1], F32, tag="bqk", name="bqk_sb")
            nc.sync.dma_start(bqk_sb[:], bqk[:])
            xlo_sb = sb.tile([128, 4, GROLL], F8, tag="xlo", name="xlo_sb")
            nc.sync.dma_start(xlo_sb[:], xlo.rearrange("(co p) f -> p co f", p=128))
            xmlo_sb = sb.tile([128, 4, GROLL], F8, tag="xmlo", name="xmlo_sb")
            nc.sync.dma_start(xmlo_sb[:], xmlo.rearrange("(co p) f -> p co f", p=128))
            for c in range(2, NCH):
                t = sb.tile([128, 4, GROLL], F8, tag=f"x{c}", name=f"x{c}")
                nc.sync.dma_start(t[:], xrr[:, :, c * GROLL:(c + 1) * GROLL])
                xc[c] = t

            # ---- persistent sbuf ----
            qk_sb = sb.tile([C8, 4, 2 * HW], F16, tag="qk", name="qk_sb")
            res_sb = sb.tile([128, NJ, 2, 2], F16, tag="res", name="res_sb")
            pam_sb = sb.tile([128, GPC, 2], F16, tag="pam", name="pam_sb")

            # ---- PE warmup (p-state ramp during DMA wait) ----
            warm_sb = sb.tile([128, 2, 2 * HW], F8, tag="warm", name="warm_sb")
            nc.gpsimd.memset(warm_sb[:], 0.0)
            wt = dpsum.tile([128, 2, 512], F32, tag="d", name="warm_ps")
            for wi in range(4):
                nc.tensor.matmul(
                    wt[:, wi % 2, 0:2 * HW], warm_sb[:, :, 0:128],
                    warm_sb[:], start=True, stop=True, perf_mode=DR,
                )

            def side_matmuls(pt, jsl, j0, j1, side, close):
                """u- or d-side matmuls for gen (j0, j1) into pt slots
                jsl. ONE start group per psum bank (a second start=True
                in a bank zeroes it): kt0 full-width starts, diag
                cross-terms accumulate sub-regions, kt1 stops."""
                for jx, j in enumerate((j0, j1)):
                    cc, jp = divmod(j, JPC)
                    out = pt[:, jsl[jx], 0:2 * HW]
                    mo = side * (GROLL // 2)
                    nc.tensor.matmul(
                        out, xc[cc][:, 0:2, jp * 128:(jp + 1) * 128],
                        xm_sb[:, 0:2, mo:mo + 2 * HW],
                        start=True, stop=False, perf_mode=DR)
                    for gp in range(2):
                        if (j, gp) not in DIAG_TILES:
                            continue
                        off, wid = DIAG_MO[(j, gp)]
                        sub = pt[:, jsl[jx], off:off + wid]
                        for lt, rt in ((xlo_sb, xm_sb), (xc[0], xmlo_sb)):
                            for kt in range(2):
                                nc.tensor.matmul(
                                    sub,
                                    lt[:, 2 * kt:2 * kt + 2,
                                       jp * 128:(jp + 1) * 128],
                                    rt[:, 2 * kt:2 * kt + 2,
                                       mo + off:mo + off + wid],
                                    start=False, stop=False,
                                    perf_mode=DR)
                    nc.tensor.matmul(
                        out, xc[cc][:, 2:4, jp * 128:(jp + 1) * 128],
                        xm_sb[:, 2:4, mo:mo + 2 * HW],
                        start=False, stop=close, perf_mode=DR)

            def d4(pt):
                return pt[:, :, 0:2 * HW].rearrange(
                    "p j (g s m) -> p j g s m", g=2, s=2)

            def e_cast(dt, j0):
                """Act: |d| -> fp16 sbuf."""
                et = epool.tile([128, 2, 2, 2, 96], F16, tag="e",
                                name=f"e_{j0}")
                nc.scalar.activation(et[:], d4(dt), ABS)
                return et

            def idadd(ut, et):
                """PE: u += |d| via identity matmul; closes the u banks."""
                for jx in range(2):
                    nc.tensor.matmul(
                        ut[:, jx, 0:2 * HW], ident_sb[:],
                        et[:, jx], start=False, stop=True,
                    )

            def funnel(src, j0, tag):
                """DVE max-funnel over one gen's 96-wide pair maxes."""
                t1 = tpool.tile([128, 2, 2, 2, 48], F16, tag="t1",
                                name=f"t1_{tag}{j0}")
                nc.vector.tensor_max(t1[:], src[:, :, :, :, 0:48],
                                     src[:, :, :, :, 48:96])
                t2 = tpool.tile([128, 2, 2, 2, 24], F16, tag="t2",
                                name=f"t2_{tag}{j0}")
                nc.vector.tensor_max(t2[:], t1[:, :, :, :, 0:24],
                                     t1[:, :, :, :, 24:48])
                t3 = tpool.tile([128, 2, 2, 2, 12], F16, tag="t3",
                                name=f"t3_{tag}{j0}")
                nc.vector.tensor_max(t3[:], t2[:, :, :, :, 0:12],
                                     t2[:, :, :, :, 12:24])
                nc.vector.reduce_max(res_sb[:, j0:j0 + 2], t3[:], axis=AX_X)

            def drain(gi, ut, et, j0):
                if gi in V3_GENS:
                    # no idadd was emitted: pairmax = u + e on DVE, funnel
                    pm = cpool.tile([128, 2, 2, 2, 96], F16, tag="uc",
                                    name=f"pm_{j0}")
                    nc.vector.scalar_tensor_tensor(
                        pm[:], d4(ut), 0.0, et[:], op0=BP, op1=ADD)
                    funnel(pm, j0, "v3")
                elif gi in V2_GENS:
                    uc = cpool.tile([128, 2, 2, 2, 96], F16, tag="uc",
                                    name=f"uc_{j0}")
                    nc.scalar.activation(uc[:], d4(ut), IDENT)
                    funnel(uc, j0, "v2")
                else:
                    nc.vector.reduce_max(res_sb[:, j0:j0 + 2], d4(ut),
                                         axis=AX_X)

            # ---- PAM (psum gens ride the d-ring) ----
            def pam_proj(qi):
                pt = dpsum.tile([128, 2, 512], F32, tag="d", name=f"proj{qi}")
                for gp in range(2):
                    for kt in range(2):
                        nc.tensor.matmul(
                            pt[0:C8, gp, 0:2 * HW],
                            w_sb[:, 2 * kt:2 * kt + 2,
                                 qi * C8:(qi + 1) * C8],
                            xc[0][:, 2 * kt:2 * kt + 2,
                                  gp * 2 * HW:(gp + 1) * 2 * HW],
                            start=(kt == 0), stop=(kt == 1), perf_mode=DR,
                        )
                nc.scalar.activation(
                    qk_sb[:, 2 * qi:2 * qi + 2], pt[0:C8, :, 0:2 * HW],
                    IDENT, bias=bqk_sb[qi * C8:(qi + 1) * C8])

            def pam_energy(h):
                """energy for images b = 2h, 2h+1 in one d-ring gen.
                Each bank is zero-initialized by one full-width matmul on
                the zeroed warm tile (one start group per bank), then the
                two m-chunk matmuls accumulate their sub-regions."""
                et = dpsum.tile([128, 2, 512], F32, tag="d", name=f"en{h}")
                for bb in range(2):
                    b = 2 * h + bb
                    gp, s = divmod(b, 2)
                    qb = qk_sb[:, 2 * 0 + gp, s * HW:(s + 1) * HW]
                    kb = qk_sb[:, 2 * 1 + gp, s * HW:(s + 1) * HW]
                    nc.tensor.matmul(
                        et[:, bb, 0:2 * HW], warm_sb[:, :, 0:128],
                        warm_sb[:], start=True, stop=False, perf_mode=DR)
                    for mch, m0 in enumerate((0, 64)):
                        nc.tensor.matmul(
                            et[:, bb, mch * HW:mch * HW + HW],
                            qb[:, m0:m0 + 128], kb[:],
                            start=False, stop=(mch == 1),
                        )
                nc.vector.reduce_max(
                    pam_sb[:, 2 * h:2 * h + 2],
                    et[:, :, 0:2 * HW].rearrange("p b (s n) -> p b s n", s=2),
                    axis=AX_X)

            # ---- emission schedule ----
            # Iteration k emits: mms(k), idadd(k-1)+drain(k-1), e-cast(k),
            # pam hook. Rings: u_k reuses u_{k-2} (drained in iter k-1);
            # d_k reuses d_{k-2} (e-cast emitted in iter k-2).
            pend = None   # (gi, ut, et, j0)
            for gi, (j0, j1) in enumerate(GEN_JS):
                dt = dpsum.tile([128, 2, 512], F32, tag="d", name=f"d_{j0}")
                side_matmuls(dt, (0, 1), j0, j1, 1, True)
                et = e_cast(dt, j0)
                ut = upsum.tile([128, 2, 512], F32, tag="u", name=f"u_{j0}")
                side_matmuls(ut, (0, 1), j0, j1, 0, gi in V3_GENS)
                if pend is not None:
                    pgi, put, pet, pj0 = pend
                    if pgi not in V3_GENS:
                        idadd(put, pet)
                    drain(pgi, put, pet, pj0)
                pend = (gi, ut, et, j0)
                if gi == 1:
                    pam_proj(0)
                elif gi == 2:
                    pam_proj(1)
                elif gi == 5:
                    pam_energy(0)
                elif gi == 6:
                    pam_energy(1)
                elif gi == 8:
                    nc.sync.dma_start(pam[:], pam_sb[:])
                elif gi == 10:
                    # gens 0-9 drained: js 6-13, 0-5, 14-19
                    nc.sync.dma_start(kres[:, 0:20], res_sb[:, 0:20])
                elif gi == 16:
                    nc.sync.dma_start(kres[:, 20:32], res_sb[:, 20:32])
                elif gi == 22:
                    nc.sync.dma_start(kres[:, 32:44], res_sb[:, 32:44])
            pgi, put, pet, pj0 = pend
            if pgi not in V3_GENS:
                idadd(put, pet)
            drain(pgi, put, pet, pj0)
            nc.sync.dma_start(kres[:, 44:NJ], res_sb[:, 44:NJ])

    nc.compile()
    return nc


def kernel(x, Wq, bq, Wk, bk):
    global _COMPILED
    if _COMPILED is None:
        _COMPILED = _build()
    nc = _COMPILED

    x = np.ascontiguousarray(x, dtype=np.float32)
    xT = x.reshape(B, C, HW).transpose(1, 0, 2).reshape(C, FLAT)
    xT8 = np.ascontiguousarray(xT).astype(ml_dtypes.float8_e4m3)
    xT8f = xT8.astype(np.float32)
    w2 = np.concatenate([np.asarray(Wq, np.float32).T,
                         np.asarray(Wk, np.float32).T], axis=1)
    w8 = np.ascontiguousarray(w2).astype(ml_dtypes.float8_e4m3)
    bqk2 = np.concatenate([np.asarray(bq, np.float32),
                           np.asarray(bk, np.float32)]).reshape(2 * C8, 1)
    bqk2 = np.ascontiguousarray(bqk2)
    id16 = np.eye(128, dtype=np.float16)

    in_maps = []
    for i in range(N_CORES):
        own = xT[:, i * GROLL:(i + 1) * GROLL].reshape(C, GPC, HW // 2, 2)
        xs = (own[..., 0] + own[..., 1]) * 0.5        # [C, 4, 96]
        xd = (own[..., 0] - own[..., 1]) * 0.5
        xmf = np.concatenate(
            [xs.reshape(C, GROLL // 2), xd.reshape(C, GROLL // 2)], axis=1)
        xm8 = xmf.astype(ml_dtypes.float8_e4m3)
        xmlo8 = (xmf - xm8.astype(np.float32)).astype(ml_dtypes.float8_e4m3)
        in_maps.append({
            "xr": np.ascontiguousarray(np.roll(xT8, -i * GROLL, axis=1)),
            "xlo": np.ascontiguousarray(
                xT[:, i * GROLL:(i + 1) * GROLL]
                - xT8f[:, i * GROLL:(i + 1) * GROLL]
            ).astype(ml_dtypes.float8_e4m3),
            "xm": np.ascontiguousarray(xm8),
            "xmlo": np.ascontiguousarray(xmlo8),
            "w": w8,
            "bqk": bqk2,
            "ident": id16,
        })

    res = run_bass_kernel_spmd(nc, in_maps, core_ids=list(range(N_CORES)))

    kernel_max = np.empty((B, FLAT), np.float32)
    pos_max = np.empty((B, HW), np.float32)
    for i, r in enumerate(res.results):
        kr = np.asarray(r["kres"]).astype(np.float32)   # [128, NJ, 2, 2]
        arr = kr.transpose(2, 3, 1, 0).reshape(GPC, FLAT)
        for gl in range(GPC):
            kernel_max[i * GPC + gl] = np.roll(arr[gl], i * GROLL)
        pm = np.asarray(r["pam"]).astype(np.float32)    # [128, 4, 2]
        for b in range(GPC):
            pos_max[i * GPC + b, 0:128] = pm[:, b, 0]
            pos_max[i * GPC + b, 128:HW] = pm[64:128, b, 1]

    return np.concatenate(
        [kernel_max.reshape(B, B, HW), pos_max[None]], axis=0
    ).astype(np.float32)
